# revision 16
# baseline (speedup 1.0000x reference)
"""Trainium2 Bass kernel for YOLO-style post-processing (nms_detection).

Contract: kernel(**inputs) takes the FULL unsharded inputs
(y_pred [32, 10647, 85] f32) and returns the full outputs matching
reference.reference():
    boxes [N,4] f32, box_scores [N,1] f32, box_classes [N,1] i32,
    selected_idx [10] i32, selected_box_scores [10] f32   (N = 340704)

Strategy:
  * Data-parallel over boxes on 8 NeuronCores (SPMD, one Bass program,
    per-core input shards; no collectives).  Each core decodes boxes,
    computes scores = conf * max(class_probs), and the exact
    first-occurrence argmax over the 80 classes, for its 42588-box shard
    (padded to 42624 = 128 partitions x 333).
  * The 10-step sequential NMS tail runs on the host over a top-K
    candidate subset with a per-iteration strict-cutoff guard that makes
    it provably identical to the full NMS (falls back to full-N NMS on
    guard failure).  Candidate box coordinates/areas are recomputed on
    host in float32 exactly as the reference does, so the selected
    outputs are bitwise-faithful to the reference semantics.

Layout per core: box b in [0, 42624) lives at partition p = b // 333,
free index f = b % 333, so both input rows (85 floats each) and output
rows are contiguous per partition for DMA.
"""

import numpy as np

import concourse.bass as bass
import concourse.bacc as bacc
import concourse.tile as tile
import concourse.mybir as mybir
from concourse.bass_utils import run_bass_kernel_spmd

F32 = mybir.dt.float32
I32 = mybir.dt.int32
ALU = mybir.AluOpType
AX = mybir.AxisListType

N_CORES = 8
P = 128

# problem constants (mirrors reference.py; kernel.py must be self-contained)
YOLO_INPUT_SIZE = np.float32(416.0)
SCORE_THRESHOLD = np.float32(0.3)
IOU_THRESHOLD = np.float32(0.45)
MAX_OUTPUT = 10

R416 = np.float32(np.float32(1.0) / np.float32(416.0))   # RN(1/416)
R416H = np.float32(R416 * np.float32(0.5))               # exact halving

NEG_BIG = np.float32(-131072.0)  # -2^17 rebias for argmax-index encoding


def _bcast_mid(ap, n):
    """[P, A] -> [P, n, A] with 0-stride middle dim."""
    return ap.unsqueeze(1).broadcast_to([ap.shape[0], n, ap.shape[1]])


def _bcast_last(ap, n):
    """[P, A] -> [P, A, n] with 0-stride last dim."""
    return ap.unsqueeze(2).broadcast_to([ap.shape[0], ap.shape[1], n])


_CUSTOM_OPS = {}


def _register_op(name, spec):
    """Register a runtime-defined custom DVE op (sha computed locally)."""
    import concourse.dve_ops as dve_ops_mod
    from concourse.dve_spec import lower as dve_lower, _has_src1
    from concourse.dve_uop import DveOpSpec

    if name in _CUSTOM_OPS:
        return _CUSTOM_OPS[name]
    shas = {}
    for ver in ("v3", "v4"):
        uops = dve_lower(spec, ver=ver)
        shas[ver] = DveOpSpec(name=name, opcode=None, uops=uops, rd1_en=True).sha(ver)
    op = dve_ops_mod.DveOp(name, spec, subdim=False, uops_sha=shas)
    if all(o.name != name for o in dve_ops_mod.OPS):
        dve_ops_mod.OPS.append(op)
        dve_ops_mod._SUB_OPCODE_FOR_NAME[name] = (
            dve_ops_mod._CUSTOM_DVE_ROW_BASE + len(dve_ops_mod.OPS) - 1
        )
    dve_ops_mod.CUSTOM_DVE_SPECS[name] = spec
    _CUSTOM_OPS[name] = op
    return op


def _register_decode_ops():
    """DECODE_LO: relu(in0*c0 - in1*c1); DECODE_HI: min(in0*c0 + in1*c1, 1).

    Each ALU stage rounds to f32, so these match the unfused op sequence
    bitwise."""
    from concourse.dve_spec import Spec, Src0, Src1, C0, C1, One, relu, minn

    r, rh = float(R416), float(R416H)

    def _lo_ref(in0, in1, c0, c1, c2):
        a = (np.asarray(in0, np.float32) * np.float32(c0)).astype(np.float32)
        b = (np.asarray(in1, np.float32) * np.float32(c1)).astype(np.float32)
        return np.maximum((a - b).astype(np.float32), np.float32(0.0))

    def _hi_ref(in0, in1, c0, c1, c2):
        a = (np.asarray(in0, np.float32) * np.float32(c0)).astype(np.float32)
        b = (np.asarray(in1, np.float32) * np.float32(c1)).astype(np.float32)
        return np.minimum((a + b).astype(np.float32), np.float32(1.0))

    lo = _register_op(
        "DECBOXLO_ANT",
        Spec(body=relu(Src0 * C0 - Src1 * C1), reference=_lo_ref),
    )
    hi = _register_op(
        "DECBOXHI_ANT",
        Spec(body=minn(Src0 * C0 + Src1 * C1, One), reference=_hi_ref),
    )
    return lo, hi


def _register_selgeidx():
    """Custom DVE op: out = (in0 >= in1) ? stream_index : s1.

    Fuses the mask + masked-index passes of the per-box argmax into one
    VectorE instruction.  s1 (C1) is a compile-time float (required when
    in1 has two free dims).
    """
    from concourse.dve_spec import Spec, Src0, Src1, C1, Idx, select

    def _ref(in0, in1, c0, c1, c2):
        in0 = np.asarray(in0, np.float32)
        fshape = in0.shape[1:]
        idx = np.arange(int(np.prod(fshape)), dtype=np.float32).reshape(fshape)
        return np.where(
            in0 >= np.asarray(in1, np.float32), idx[None], np.float32(c1)
        ).astype(np.float32)

    return _register_op(
        "SELGEIDX_ANT", Spec(body=select(Src0 >= Src1, Idx, C1), reference=_ref)
    )


def build_program(fpad=333, fg=37, n_cores=N_CORES, use_custom=True, bufs=(3, 2, 3)):
    """Build + compile the SPMD Bass program (one NeuronCore's view).

    fpad:     boxes per partition (per core total = 128*fpad)
    fg:       boxes per partition processed per tile (fpad % fg == 0)
    use_custom: fuse passes into custom DVE ops
    """
    assert fpad % fg == 0
    ntiles = fpad // fg
    bpad = P * fpad
    sel_op = _register_selgeidx() if use_custom else None
    lo_op, hi_op = _register_decode_ops() if use_custom else (None, None)

    nc = bacc.Bacc(
        "TRN2",
        target_bir_lowering=False,
        debug=False,
        num_devices=n_cores,
    )

    yp = nc.dram_tensor("yp", [bpad, 85], F32, kind="ExternalInput")
    boxes_o = nc.dram_tensor("boxes", [bpad, 4], F32, kind="ExternalOutput")
    scores_o = nc.dram_tensor("scores", [bpad], F32, kind="ExternalOutput")
    classes_o = nc.dram_tensor("classes", [bpad], I32, kind="ExternalOutput")

    ypr = yp.ap().rearrange("(p f) c -> p f c", p=P)
    boxr = boxes_o.ap().rearrange("(p f) c -> p f c", p=P)
    scorer = scores_o.ap().rearrange("(p f) -> p f", p=P)
    classr = classes_o.ap().rearrange("(p f) -> p f", p=P)

    from contextlib import ExitStack

    with tile.TileContext(nc) as tc, ExitStack() as ctx:
        persist = ctx.enter_context(tc.tile_pool(name="persist", bufs=1))
        inpool = ctx.enter_context(tc.tile_pool(name="inp", bufs=bufs[0]))
        maskpool = ctx.enter_context(tc.tile_pool(name="mask", bufs=bufs[1]))
        small = ctx.enter_context(tc.tile_pool(name="small", bufs=bufs[2]))
        dec = ctx.enter_context(tc.tile_pool(name="dec", bufs=3))

        # persistent result tiles (DMA'd out once at the end)
        boxes_sb = persist.tile([P, fpad, 4], F32)
        scores_sb = persist.tile([P, fpad], F32)
        classes_sb = persist.tile([P, fpad], I32)

        # constants (f32 iota is exact for |v| < 2^24)
        # bigiota[c] = c - 2^17: mask*bigiota -> min = argmax - 2^17
        bigiota = persist.tile([P, 80], F32)
        nc.gpsimd.iota(
            bigiota[:], pattern=[[1, 80]], base=-131072, channel_multiplier=0,
            allow_small_or_imprecise_dtypes=True,
        )
        # col80[j] = 80*j for j in [0, fg): recovers the class from the
        # custom op's per-instruction stream index (local to each tile)
        col80 = persist.tile([P, fg], F32)
        nc.gpsimd.iota(
            col80[:], pattern=[[80, fg]], base=0, channel_multiplier=0,
            allow_small_or_imprecise_dtypes=True,
        )

        for t in range(ntiles):
            sl = slice(t * fg, (t + 1) * fg)

            bt = inpool.tile([P, fg, 85], F32)
            nc.sync.dma_start(out=bt[:], in_=ypr[:, sl, :])

            probs = bt[:, :, 5:85]
            conf = bt[:, :, 4]

            # ---- class max + scores ----
            m = small.tile([P, fg], F32, tag="m")
            nc.vector.reduce_max(m[:], probs, axis=AX.X)
            nc.vector.tensor_tensor(
                out=scores_sb[:, sl], in0=conf, in1=m[:], op=ALU.mult
            )

            # ---- first-occurrence argmax over 80 classes ----
            mk = maskpool.tile([P, fg, 80], F32, tag="mk")
            tmin = small.tile([P, fg], F32, tag="tmin")
            if use_custom:
                # one fused pass: idx where p >= m else BIG
                nc.vector._custom_dve(
                    sel_op, out=mk[:], in0=probs,
                    in1=_bcast_last(m[:], 80), s1=1.0e9,
                )
                nc.vector.tensor_reduce(tmin[:], mk[:], axis=AX.X, op=ALU.min)
                # class = gmin - 80*f  (exact ints in f32; i32 on write)
                nc.vector.tensor_tensor(
                    out=classes_sb[:, sl], in0=tmin[:], in1=col80[:],
                    op=ALU.subtract,
                )
            else:
                nc.vector.tensor_tensor(
                    out=mk[:], in0=probs, in1=_bcast_last(m[:], 80),
                    op=ALU.is_ge,
                )
                nc.vector.tensor_tensor(
                    out=mk[:], in0=mk[:], in1=_bcast_mid(bigiota[:], fg),
                    op=ALU.mult,
                )
                nc.vector.tensor_reduce(tmin[:], mk[:], axis=AX.X, op=ALU.min)
                nc.vector.tensor_scalar_add(
                    out=classes_sb[:, sl], in0=tmin[:], scalar1=float(-NEG_BIG)
                )

            # ---- box decode ----
            if use_custom:
                nc.vector._custom_dve(
                    lo_op, out=boxes_sb[:, sl, 0:2], in0=bt[:, :, 0:2],
                    in1=bt[:, :, 2:4], s0=float(R416), s1=float(R416H),
                )
                nc.vector._custom_dve(
                    hi_op, out=boxes_sb[:, sl, 2:4], in0=bt[:, :, 0:2],
                    in1=bt[:, :, 2:4], s0=float(R416), s1=float(R416H),
                )
            else:
                xy = dec.tile([P, fg, 2], F32, tag="xy")
                wh = dec.tile([P, fg, 2], F32, tag="wh")
                nc.scalar.mul(xy[:], bt[:, :, 0:2], float(R416))
                nc.scalar.mul(wh[:], bt[:, :, 2:4], float(R416H))
                t1 = dec.tile([P, fg, 2], F32, tag="t1")
                t2 = dec.tile([P, fg, 2], F32, tag="t2")
                nc.vector.tensor_sub(t1[:], xy[:], wh[:])
                nc.vector.tensor_scalar_max(
                    out=boxes_sb[:, sl, 0:2], in0=t1[:], scalar1=0.0
                )
                nc.vector.tensor_add(t2[:], xy[:], wh[:])
                nc.vector.tensor_scalar_min(
                    out=boxes_sb[:, sl, 2:4], in0=t2[:], scalar1=1.0
                )

        nc.sync.dma_start(out=boxr[:], in_=boxes_sb[:])
        nc.sync.dma_start(out=scorer[:], in_=scores_sb[:])
        nc.sync.dma_start(out=classr[:], in_=classes_sb[:])

    nc.compile()
    return nc


# ---------------------------------------------------------------------------
# host-side NMS tail (bitwise-faithful to reference semantics in float32)
# ---------------------------------------------------------------------------

def _decode_boxes_np(raw):
    b = np.clip(raw / YOLO_INPUT_SIZE, np.float32(0.0), np.float32(1.0)).astype(
        np.float32
    )
    x, y, w, h = b[:, 0], b[:, 1], b[:, 2], b[:, 3]
    half = np.float32(0.5)
    one = np.float32(1.0)
    zero = np.float32(0.0)
    x1 = np.clip(x - half * w, zero, one)
    y1 = np.clip(y - half * h, zero, one)
    x2 = np.clip(x + half * w, zero, one)
    y2 = np.clip(y + half * h, zero, one)
    return np.stack([x1, y1, x2, y2], axis=-1).astype(np.float32)


def _nms_on_subset(cand_idx, boxes_c, scores_c, cutoff, n_total):
    """Run the reference NMS restricted to candidate boxes.

    cand_idx: global indices (int64) of candidates, boxes_c [K,4] f32,
    scores_c [K] f32 (exact reference-path values), cutoff: min original
    score over candidates (f32).  Returns (sel_idx[10] i32,
    sel_scores[10] f32, ok flag).  ok=False => guard failed, caller must
    fall back to the full computation.
    """
    # order candidates by global index so np.argmax tie-breaks identically
    order = np.argsort(cand_idx, kind="stable")
    cand_idx = cand_idx[order]
    boxes_c = boxes_c[order]
    scores_c = scores_c[order]

    x1, y1, x2, y2 = boxes_c[:, 0], boxes_c[:, 1], boxes_c[:, 2], boxes_c[:, 3]
    areas = ((x2 - x1) * (y2 - y1)).astype(np.float32)
    neg_inf = np.float32(-np.inf)
    sw = np.where(scores_c >= SCORE_THRESHOLD, scores_c, neg_inf).astype(np.float32)

    sel_idx = np.full(MAX_OUTPUT, -1, np.int32)
    sel_sc = np.zeros(MAX_OUTPUT, np.float32)
    for i in range(MAX_OUTPUT):
        j = int(np.argmax(sw))
        valid = np.isfinite(sw[j])
        if not valid:
            # all remaining -inf; matches reference (idx -1, score 0).
            # Guard: the true NMS could still have valid boxes outside the
            # candidate set only if cutoff >= threshold.
            if cutoff >= SCORE_THRESHOLD:
                return sel_idx, sel_sc, False
            continue
        # guard: winner must be strictly above every non-candidate score
        if not (sw[j] > cutoff):
            return sel_idx, sel_sc, False
        sel_idx[i] = np.int32(cand_idx[j])
        sel_sc[i] = scores_c[j]
        iw = np.maximum(
            np.minimum(x2, x2[j]) - np.maximum(x1, x1[j]), np.float32(0.0)
        ).astype(np.float32)
        ih = np.maximum(
            np.minimum(y2, y2[j]) - np.maximum(y1, y1[j]), np.float32(0.0)
        ).astype(np.float32)
        inter = (iw * ih).astype(np.float32)
        union = np.maximum(
            (areas + areas[j]).astype(np.float32) - inter, np.float32(1e-9)
        ).astype(np.float32)
        iou = (inter / union).astype(np.float32)
        sw = np.where(np.isfinite(sw[j]) & (iou > IOU_THRESHOLD), neg_inf, sw)
        sw[j] = neg_inf
    return sel_idx, sel_sc, True


def _host_nms_tail(flat, scores_full, topk=4096):
    """flat: y_pred reshaped [N, 85] f32; scores_full [N] f32 (device).

    Recomputes candidate boxes/scores exactly as the reference does
    (true f32 division, clips) so selected outputs match bitwise.
    """
    n = flat.shape[0]
    k = min(topk, n)
    cand = np.argpartition(scores_full, n - k)[n - k:]
    cutoff = np.float32(scores_full[cand].min())

    rows = flat[cand]
    boxes_c = _decode_boxes_np(rows[:, :4])
    scores_c = (
        rows[:, 4].astype(np.float32)
        * np.max(rows[:, 5:], axis=-1).astype(np.float32)
    ).astype(np.float32)

    global LAST_NMS_FALLBACK
    sel_idx, sel_sc, ok = _nms_on_subset(
        cand.astype(np.int64), boxes_c, scores_c, cutoff, n
    )
    LAST_NMS_FALLBACK = not ok
    if ok:
        return sel_idx, sel_sc

    # fallback: exact full-N NMS on host (never expected to trigger)
    boxes_f = _decode_boxes_np(flat[:, :4])
    scores_f = (
        flat[:, 4].astype(np.float32)
        * np.max(flat[:, 5:], axis=-1).astype(np.float32)
    ).astype(np.float32)
    sel_idx, sel_sc, ok = _nms_on_subset(
        np.arange(n, dtype=np.int64), boxes_f, scores_f, np.float32(-np.inf), n
    )
    assert ok
    return sel_idx, sel_sc


# ---------------------------------------------------------------------------
# public entry point
# ---------------------------------------------------------------------------

_NC_CACHE = {}
LAST_NMS_FALLBACK = False


def _get_program():
    key = "main"
    if key not in _NC_CACHE:
        _NC_CACHE[key] = build_program()
    return _NC_CACHE[key]


def run_device(flat: np.ndarray, trace: bool = False, **kwargs):
    """Run the SPMD device program on the flattened [N, 85] input.

    Returns (boxes [N,4], scores [N], classes [N], BassKernelResults).
    """
    n = flat.shape[0]
    per_core = n // N_CORES                         # 42588
    fpad = 333
    bpad = P * fpad                                 # 42624

    nc = _get_program()

    in_maps = []
    for c in range(N_CORES):
        shard = flat[c * per_core:(c + 1) * per_core]
        if bpad != per_core:
            pad = np.zeros((bpad - per_core, 85), np.float32)
            shard = np.concatenate([shard, pad], axis=0)
        in_maps.append({"yp": np.ascontiguousarray(shard)})

    res = run_bass_kernel_spmd(
        nc, in_maps, core_ids=list(range(N_CORES)), trace=trace, **kwargs
    )
    results = res.results

    boxes = np.concatenate(
        [results[c]["boxes"][:per_core] for c in range(N_CORES)], axis=0
    ).astype(np.float32)
    scores = np.concatenate(
        [results[c]["scores"][:per_core] for c in range(N_CORES)], axis=0
    ).astype(np.float32)
    classes = np.concatenate(
        [results[c]["classes"][:per_core] for c in range(N_CORES)], axis=0
    ).astype(np.int32)
    return boxes, scores, classes, res


def kernel(y_pred: np.ndarray):
    y_pred = np.asarray(y_pred, dtype=np.float32)
    n = y_pred.shape[0] * y_pred.shape[1]          # 340704
    flat = np.ascontiguousarray(y_pred.reshape(n, y_pred.shape[-1]))

    boxes, scores, classes, _ = run_device(flat)

    sel_idx, sel_sc = _host_nms_tail(flat, scores)

    return (
        boxes,
        scores[:, None],
        classes[:, None],
        sel_idx,
        sel_sc,
    )


if __name__ == "__main__":
    rng = np.random.default_rng(0)
    y = rng.random((32, 10647, 85), dtype=np.float32) * np.array(
        [416.0] * 4 + [1.0] * 81, np.float32
    )
    out = kernel(y_pred=y)
    for o in out:
        print(o.shape, o.dtype)


# revision 20
# speedup vs baseline: 1.1188x; 1.1188x over previous
"""Trainium2 Bass kernel for YOLO-style post-processing (nms_detection).

Contract: kernel(**inputs) takes the FULL unsharded inputs
(y_pred [32, 10647, 85] f32) and returns the full outputs matching
reference.reference():
    boxes [N,4] f32, box_scores [N,1] f32, box_classes [N,1] i32,
    selected_idx [10] i32, selected_box_scores [10] f32   (N = 340704)

Strategy:
  * Data-parallel over boxes on 8 NeuronCores (SPMD, one Bass program,
    per-core input shards; no collectives).  Each core decodes boxes,
    computes scores = conf * max(class_probs), and the exact
    first-occurrence argmax over the 80 classes, for its 42588-box shard
    (padded to 42624 = 128 partitions x 333).
  * The 10-step sequential NMS tail runs on the host over a top-K
    candidate subset with a per-iteration strict-cutoff guard that makes
    it provably identical to the full NMS (falls back to full-N NMS on
    guard failure).  Candidate box coordinates/areas are recomputed on
    host in float32 exactly as the reference does, so the selected
    outputs are bitwise-faithful to the reference semantics.

Layout per core: box b in [0, 42624) lives at partition p = b // 333,
free index f = b % 333, so both input rows (85 floats each) and output
rows are contiguous per partition for DMA.
"""

import numpy as np

import concourse.bass as bass
import concourse.bacc as bacc
import concourse.tile as tile
import concourse.mybir as mybir
from concourse.bass_utils import run_bass_kernel_spmd

F32 = mybir.dt.float32
I32 = mybir.dt.int32
ALU = mybir.AluOpType
AX = mybir.AxisListType

N_CORES = 8
P = 128

# problem constants (mirrors reference.py; kernel.py must be self-contained)
YOLO_INPUT_SIZE = np.float32(416.0)
SCORE_THRESHOLD = np.float32(0.3)
IOU_THRESHOLD = np.float32(0.45)
MAX_OUTPUT = 10

R416 = np.float32(np.float32(1.0) / np.float32(416.0))   # RN(1/416)
R416H = np.float32(R416 * np.float32(0.5))               # exact halving

NEG_BIG = np.float32(-131072.0)  # -2^17 rebias for argmax-index encoding


def _bcast_mid(ap, n):
    """[P, A] -> [P, n, A] with 0-stride middle dim."""
    return ap.unsqueeze(1).broadcast_to([ap.shape[0], n, ap.shape[1]])


def _bcast_last(ap, n):
    """[P, A] -> [P, A, n] with 0-stride last dim."""
    return ap.unsqueeze(2).broadcast_to([ap.shape[0], ap.shape[1], n])


_CUSTOM_OPS = {}


def _register_op(name, spec, subdim=False):
    """Register a runtime-defined custom DVE op (sha computed locally)."""
    import concourse.dve_ops as dve_ops_mod
    from concourse.dve_spec import lower as dve_lower, _has_src1
    from concourse.dve_uop import DveOpSpec

    if name in _CUSTOM_OPS:
        return _CUSTOM_OPS[name]
    shas = {}
    for ver in ("v3", "v4"):
        uops = dve_lower(spec, ver=ver)
        shas[ver] = DveOpSpec(name=name, opcode=None, uops=uops, rd1_en=True).sha(ver)
    op = dve_ops_mod.DveOp(name, spec, subdim=subdim, uops_sha=shas)
    if all(o.name != name for o in dve_ops_mod.OPS):
        dve_ops_mod.OPS.append(op)
        dve_ops_mod._SUB_OPCODE_FOR_NAME[name] = (
            dve_ops_mod._CUSTOM_DVE_ROW_BASE + len(dve_ops_mod.OPS) - 1
        )
    dve_ops_mod.CUSTOM_DVE_SPECS[name] = spec
    _CUSTOM_OPS[name] = op
    return op


def _register_decode_ops():
    """DECODE_LO: relu(in0*c0 - in1*c1); DECODE_HI: min(in0*c0 + in1*c1, 1).

    Each ALU stage rounds to f32, so these match the unfused op sequence
    bitwise."""
    from concourse.dve_spec import Spec, Src0, Src1, C0, C1, One, relu, minn

    r, rh = float(R416), float(R416H)

    def _lo_ref(in0, in1, c0, c1, c2):
        a = (np.asarray(in0, np.float32) * np.float32(c0)).astype(np.float32)
        b = (np.asarray(in1, np.float32) * np.float32(c1)).astype(np.float32)
        return np.maximum((a - b).astype(np.float32), np.float32(0.0))

    def _hi_ref(in0, in1, c0, c1, c2):
        a = (np.asarray(in0, np.float32) * np.float32(c0)).astype(np.float32)
        b = (np.asarray(in1, np.float32) * np.float32(c1)).astype(np.float32)
        return np.minimum((a + b).astype(np.float32), np.float32(1.0))

    lo = _register_op(
        "DECBOXLO_ANT",
        Spec(body=relu(Src0 * C0 - Src1 * C1), reference=_lo_ref),
    )
    hi = _register_op(
        "DECBOXHI_ANT",
        Spec(body=minn(Src0 * C0 + Src1 * C1, One), reference=_hi_ref),
    )
    return lo, hi


def _register_selgec():
    """Custom DVE op: out = (in0 >= in1) ? page-local index : s1.

    subdim op: in0/in1 are [P, S, N]; PageIdx(Zero, C0) holds 80*s (s0
    must be passed as float(N)), so Idx - pg is the within-page position.
    Output fits f16 exactly (values in [0, 79] or the fill 1024)."""
    from concourse.dve_spec import Spec, Src0, Src1, C0, C1, Zero, Idx, PageIdx, select

    def _ref(in0, in1, c0, c1, c2):
        in0 = np.asarray(in0, np.float32)
        assert in0.ndim == 3
        idx = np.arange(in0.shape[2], dtype=np.float32)[None, None, :]
        return np.where(
            in0 >= np.asarray(in1, np.float32), idx, np.float32(c1)
        ).astype(np.float32)

    spec = Spec(
        body=select(Src0 >= Src1, Idx - PageIdx(Zero, C0), C1), reference=_ref
    )
    return _register_op("SELGEC_ANT", spec, subdim=True)


def _register_selgeidx():
    """Custom DVE op: out = (in0 >= in1) ? stream_index : s1.

    Fuses the mask + masked-index passes of the per-box argmax into one
    VectorE instruction.  s1 (C1) is a compile-time float (required when
    in1 has two free dims).
    """
    from concourse.dve_spec import Spec, Src0, Src1, C1, Idx, select

    def _ref(in0, in1, c0, c1, c2):
        in0 = np.asarray(in0, np.float32)
        fshape = in0.shape[1:]
        idx = np.arange(int(np.prod(fshape)), dtype=np.float32).reshape(fshape)
        return np.where(
            in0 >= np.asarray(in1, np.float32), idx[None], np.float32(c1)
        ).astype(np.float32)

    return _register_op(
        "SELGEIDX_ANT", Spec(body=select(Src0 >= Src1, Idx, C1), reference=_ref)
    )


def build_program(
    fpad=333, fg=37, n_cores=N_CORES, use_custom=True, bufs=(3, 2, 3),
    argmax_mode="selgec",
):
    """Build + compile the SPMD Bass program (one NeuronCore's view).

    fpad:     boxes per partition (per core total = 128*fpad)
    fg:       boxes per partition processed per tile (fpad % fg == 0)
    use_custom: fuse passes into custom DVE ops
    argmax_mode: "selgec" (page-local f16 + tree-min) or "selgeidx"
    """
    assert fpad % fg == 0
    ntiles = fpad // fg
    bpad = P * fpad
    F16 = mybir.dt.float16
    if not use_custom:
        argmax_mode = "stock"
    sel_op = _register_selgeidx() if use_custom else None
    selc_op = _register_selgec() if use_custom else None
    lo_op, hi_op = _register_decode_ops() if use_custom else (None, None)

    nc = bacc.Bacc(
        "TRN2",
        target_bir_lowering=False,
        debug=False,
        num_devices=n_cores,
    )

    yp = nc.dram_tensor("yp", [bpad, 85], F32, kind="ExternalInput")
    boxes_o = nc.dram_tensor("boxes", [bpad, 4], F32, kind="ExternalOutput")
    scores_o = nc.dram_tensor("scores", [bpad], F32, kind="ExternalOutput")
    classes_o = nc.dram_tensor("classes", [bpad], I32, kind="ExternalOutput")

    ypr = yp.ap().rearrange("(p f) c -> p f c", p=P)
    boxr = boxes_o.ap().rearrange("(p f) c -> p f c", p=P)
    scorer = scores_o.ap().rearrange("(p f) -> p f", p=P)
    classr = classes_o.ap().rearrange("(p f) -> p f", p=P)

    from contextlib import ExitStack

    with tile.TileContext(nc) as tc, ExitStack() as ctx:
        persist = ctx.enter_context(tc.tile_pool(name="persist", bufs=1))
        inpool = ctx.enter_context(tc.tile_pool(name="inp", bufs=bufs[0]))
        maskpool = ctx.enter_context(tc.tile_pool(name="mask", bufs=bufs[1]))
        small = ctx.enter_context(tc.tile_pool(name="small", bufs=bufs[2]))
        dec = ctx.enter_context(tc.tile_pool(name="dec", bufs=3))

        # persistent result tiles (DMA'd out once at the end)
        boxes_sb = persist.tile([P, fpad, 4], F32)
        scores_sb = persist.tile([P, fpad], F32)
        classes_sb = persist.tile([P, fpad], I32)

        # constants (f32 iota is exact for |v| < 2^24)
        # bigiota[c] = c - 2^17: mask*bigiota -> min = argmax - 2^17
        bigiota = persist.tile([P, 80], F32)
        nc.gpsimd.iota(
            bigiota[:], pattern=[[1, 80]], base=-131072, channel_multiplier=0,
            allow_small_or_imprecise_dtypes=True,
        )
        # col80[j] = 80*j for j in [0, fg): recovers the class from the
        # custom op's per-instruction stream index (local to each tile)
        col80 = persist.tile([P, fg], F32)
        nc.gpsimd.iota(
            col80[:], pattern=[[80, fg]], base=0, channel_multiplier=0,
            allow_small_or_imprecise_dtypes=True,
        )

        for t in range(ntiles):
            sl = slice(t * fg, (t + 1) * fg)

            bt = inpool.tile([P, fg, 85], F32)
            nc.sync.dma_start(out=bt[:], in_=ypr[:, sl, :])

            probs = bt[:, :, 5:85]
            conf = bt[:, :, 4]

            # ---- class max + scores ----
            m = small.tile([P, fg], F32, tag="m")
            nc.vector.reduce_max(m[:], probs, axis=AX.X)
            nc.vector.tensor_tensor(
                out=scores_sb[:, sl], in0=conf, in1=m[:], op=ALU.mult
            )

            # ---- first-occurrence argmax over 80 classes ----
            if argmax_mode == "selgec":
                # fused pass -> page-local index in f16, then a 2x-mode
                # f16 min-tree (cheaper than the 1x tensor_reduce)
                mk16 = maskpool.tile([P, fg, 80], F16, tag="mk16")
                nc.vector._custom_dve(
                    selc_op, out=mk16[:], in0=probs,
                    in1=_bcast_last(m[:], 80), s0=80.0, s1=1024.0,
                )
                cur = mk16[:]
                for w in (40, 20, 10, 5):
                    dst = dec.tile([P, fg, w], F16, tag=f"tm{w}")
                    nc.vector.tensor_tensor(
                        out=dst[:], in0=cur[:, :, 0:w], in1=cur[:, :, w:2 * w],
                        op=ALU.min,
                    )
                    cur = dst[:]
                tmin16 = small.tile([P, fg], F16, tag="tmin16")
                nc.vector.tensor_reduce(tmin16[:], cur, axis=AX.X, op=ALU.min)
                nc.vector.tensor_scalar_add(
                    out=classes_sb[:, sl], in0=tmin16[:], scalar1=0.0
                )
                mk = None
                tmin = None
            else:
                mk = maskpool.tile([P, fg, 80], F32, tag="mk")
                tmin = small.tile([P, fg], F32, tag="tmin")
            if argmax_mode == "selgec":
                pass
            elif use_custom:
                # one fused pass: idx where p >= m else BIG
                nc.vector._custom_dve(
                    sel_op, out=mk[:], in0=probs,
                    in1=_bcast_last(m[:], 80), s1=1.0e9,
                )
                nc.vector.tensor_reduce(tmin[:], mk[:], axis=AX.X, op=ALU.min)
                # class = gmin - 80*f  (exact ints in f32; i32 on write)
                nc.vector.tensor_tensor(
                    out=classes_sb[:, sl], in0=tmin[:], in1=col80[:],
                    op=ALU.subtract,
                )
            else:
                nc.vector.tensor_tensor(
                    out=mk[:], in0=probs, in1=_bcast_last(m[:], 80),
                    op=ALU.is_ge,
                )
                nc.vector.tensor_tensor(
                    out=mk[:], in0=mk[:], in1=_bcast_mid(bigiota[:], fg),
                    op=ALU.mult,
                )
                nc.vector.tensor_reduce(tmin[:], mk[:], axis=AX.X, op=ALU.min)
                nc.vector.tensor_scalar_add(
                    out=classes_sb[:, sl], in0=tmin[:], scalar1=float(-NEG_BIG)
                )

            # ---- box decode ----
            if use_custom:
                nc.vector._custom_dve(
                    lo_op, out=boxes_sb[:, sl, 0:2], in0=bt[:, :, 0:2],
                    in1=bt[:, :, 2:4], s0=float(R416), s1=float(R416H),
                )
                nc.vector._custom_dve(
                    hi_op, out=boxes_sb[:, sl, 2:4], in0=bt[:, :, 0:2],
                    in1=bt[:, :, 2:4], s0=float(R416), s1=float(R416H),
                )
            else:
                xy = dec.tile([P, fg, 2], F32, tag="xy")
                wh = dec.tile([P, fg, 2], F32, tag="wh")
                nc.scalar.mul(xy[:], bt[:, :, 0:2], float(R416))
                nc.scalar.mul(wh[:], bt[:, :, 2:4], float(R416H))
                t1 = dec.tile([P, fg, 2], F32, tag="t1")
                t2 = dec.tile([P, fg, 2], F32, tag="t2")
                nc.vector.tensor_sub(t1[:], xy[:], wh[:])
                nc.vector.tensor_scalar_max(
                    out=boxes_sb[:, sl, 0:2], in0=t1[:], scalar1=0.0
                )
                nc.vector.tensor_add(t2[:], xy[:], wh[:])
                nc.vector.tensor_scalar_min(
                    out=boxes_sb[:, sl, 2:4], in0=t2[:], scalar1=1.0
                )

        nc.sync.dma_start(out=boxr[:], in_=boxes_sb[:])
        nc.sync.dma_start(out=scorer[:], in_=scores_sb[:])
        nc.sync.dma_start(out=classr[:], in_=classes_sb[:])

    nc.compile()
    return nc


# ---------------------------------------------------------------------------
# host-side NMS tail (bitwise-faithful to reference semantics in float32)
# ---------------------------------------------------------------------------

def _decode_boxes_np(raw):
    b = np.clip(raw / YOLO_INPUT_SIZE, np.float32(0.0), np.float32(1.0)).astype(
        np.float32
    )
    x, y, w, h = b[:, 0], b[:, 1], b[:, 2], b[:, 3]
    half = np.float32(0.5)
    one = np.float32(1.0)
    zero = np.float32(0.0)
    x1 = np.clip(x - half * w, zero, one)
    y1 = np.clip(y - half * h, zero, one)
    x2 = np.clip(x + half * w, zero, one)
    y2 = np.clip(y + half * h, zero, one)
    return np.stack([x1, y1, x2, y2], axis=-1).astype(np.float32)


def _nms_on_subset(cand_idx, boxes_c, scores_c, cutoff, n_total):
    """Run the reference NMS restricted to candidate boxes.

    cand_idx: global indices (int64) of candidates, boxes_c [K,4] f32,
    scores_c [K] f32 (exact reference-path values), cutoff: min original
    score over candidates (f32).  Returns (sel_idx[10] i32,
    sel_scores[10] f32, ok flag).  ok=False => guard failed, caller must
    fall back to the full computation.
    """
    # order candidates by global index so np.argmax tie-breaks identically
    order = np.argsort(cand_idx, kind="stable")
    cand_idx = cand_idx[order]
    boxes_c = boxes_c[order]
    scores_c = scores_c[order]

    x1, y1, x2, y2 = boxes_c[:, 0], boxes_c[:, 1], boxes_c[:, 2], boxes_c[:, 3]
    areas = ((x2 - x1) * (y2 - y1)).astype(np.float32)
    neg_inf = np.float32(-np.inf)
    sw = np.where(scores_c >= SCORE_THRESHOLD, scores_c, neg_inf).astype(np.float32)

    sel_idx = np.full(MAX_OUTPUT, -1, np.int32)
    sel_sc = np.zeros(MAX_OUTPUT, np.float32)
    for i in range(MAX_OUTPUT):
        j = int(np.argmax(sw))
        valid = np.isfinite(sw[j])
        if not valid:
            # all remaining -inf; matches reference (idx -1, score 0).
            # Guard: the true NMS could still have valid boxes outside the
            # candidate set only if cutoff >= threshold.
            if cutoff >= SCORE_THRESHOLD:
                return sel_idx, sel_sc, False
            continue
        # guard: winner must be strictly above every non-candidate score
        if not (sw[j] > cutoff):
            return sel_idx, sel_sc, False
        sel_idx[i] = np.int32(cand_idx[j])
        sel_sc[i] = scores_c[j]
        iw = np.maximum(
            np.minimum(x2, x2[j]) - np.maximum(x1, x1[j]), np.float32(0.0)
        ).astype(np.float32)
        ih = np.maximum(
            np.minimum(y2, y2[j]) - np.maximum(y1, y1[j]), np.float32(0.0)
        ).astype(np.float32)
        inter = (iw * ih).astype(np.float32)
        union = np.maximum(
            (areas + areas[j]).astype(np.float32) - inter, np.float32(1e-9)
        ).astype(np.float32)
        iou = (inter / union).astype(np.float32)
        sw = np.where(np.isfinite(sw[j]) & (iou > IOU_THRESHOLD), neg_inf, sw)
        sw[j] = neg_inf
    return sel_idx, sel_sc, True


def _host_nms_tail(flat, scores_full, topk=4096):
    """flat: y_pred reshaped [N, 85] f32; scores_full [N] f32 (device).

    Recomputes candidate boxes/scores exactly as the reference does
    (true f32 division, clips) so selected outputs match bitwise.
    """
    n = flat.shape[0]
    k = min(topk, n)
    cand = np.argpartition(scores_full, n - k)[n - k:]
    cutoff = np.float32(scores_full[cand].min())

    rows = flat[cand]
    boxes_c = _decode_boxes_np(rows[:, :4])
    scores_c = (
        rows[:, 4].astype(np.float32)
        * np.max(rows[:, 5:], axis=-1).astype(np.float32)
    ).astype(np.float32)

    global LAST_NMS_FALLBACK
    sel_idx, sel_sc, ok = _nms_on_subset(
        cand.astype(np.int64), boxes_c, scores_c, cutoff, n
    )
    LAST_NMS_FALLBACK = not ok
    if ok:
        return sel_idx, sel_sc

    # fallback: exact full-N NMS on host (never expected to trigger)
    boxes_f = _decode_boxes_np(flat[:, :4])
    scores_f = (
        flat[:, 4].astype(np.float32)
        * np.max(flat[:, 5:], axis=-1).astype(np.float32)
    ).astype(np.float32)
    sel_idx, sel_sc, ok = _nms_on_subset(
        np.arange(n, dtype=np.int64), boxes_f, scores_f, np.float32(-np.inf), n
    )
    assert ok
    return sel_idx, sel_sc


# ---------------------------------------------------------------------------
# public entry point
# ---------------------------------------------------------------------------

_NC_CACHE = {}
LAST_NMS_FALLBACK = False


def _get_program():
    key = "main"
    if key not in _NC_CACHE:
        _NC_CACHE[key] = build_program()
    return _NC_CACHE[key]


def run_device(flat: np.ndarray, trace: bool = False, **kwargs):
    """Run the SPMD device program on the flattened [N, 85] input.

    Returns (boxes [N,4], scores [N], classes [N], BassKernelResults).
    """
    n = flat.shape[0]
    per_core = n // N_CORES                         # 42588
    fpad = 333
    bpad = P * fpad                                 # 42624

    nc = _get_program()

    in_maps = []
    for c in range(N_CORES):
        shard = flat[c * per_core:(c + 1) * per_core]
        if bpad != per_core:
            pad = np.zeros((bpad - per_core, 85), np.float32)
            shard = np.concatenate([shard, pad], axis=0)
        in_maps.append({"yp": np.ascontiguousarray(shard)})

    res = run_bass_kernel_spmd(
        nc, in_maps, core_ids=list(range(N_CORES)), trace=trace, **kwargs
    )
    results = res.results

    boxes = np.concatenate(
        [results[c]["boxes"][:per_core] for c in range(N_CORES)], axis=0
    ).astype(np.float32)
    scores = np.concatenate(
        [results[c]["scores"][:per_core] for c in range(N_CORES)], axis=0
    ).astype(np.float32)
    classes = np.concatenate(
        [results[c]["classes"][:per_core] for c in range(N_CORES)], axis=0
    ).astype(np.int32)
    return boxes, scores, classes, res


def kernel(y_pred: np.ndarray):
    y_pred = np.asarray(y_pred, dtype=np.float32)
    n = y_pred.shape[0] * y_pred.shape[1]          # 340704
    flat = np.ascontiguousarray(y_pred.reshape(n, y_pred.shape[-1]))

    boxes, scores, classes, _ = run_device(flat)

    sel_idx, sel_sc = _host_nms_tail(flat, scores)

    return (
        boxes,
        scores[:, None],
        classes[:, None],
        sel_idx,
        sel_sc,
    )


if __name__ == "__main__":
    rng = np.random.default_rng(0)
    y = rng.random((32, 10647, 85), dtype=np.float32) * np.array(
        [416.0] * 4 + [1.0] * 81, np.float32
    )
    out = kernel(y_pred=y)
    for o in out:
        print(o.shape, o.dtype)


# revision 35
# speedup vs baseline: 1.1220x; 1.0029x over previous
"""Trainium2 Bass kernel for YOLO-style post-processing (nms_detection).

Contract: kernel(**inputs) takes the FULL unsharded inputs
(y_pred [32, 10647, 85] f32) and returns the full outputs matching
reference.reference():
    boxes [N,4] f32, box_scores [N,1] f32, box_classes [N,1] i32,
    selected_idx [10] i32, selected_box_scores [10] f32   (N = 340704)

Strategy:
  * Data-parallel over boxes on 8 NeuronCores (SPMD, one Bass program,
    per-core input shards; no collectives).  Each core decodes boxes,
    computes scores = conf * max(class_probs), and the exact
    first-occurrence argmax over the 80 classes, for its 42588-box shard
    (padded to 42624 = 128 partitions x 333).
  * The 10-step sequential NMS tail runs on the host over a top-K
    candidate subset with a per-iteration strict-cutoff guard that makes
    it provably identical to the full NMS (falls back to full-N NMS on
    guard failure).  Candidate box coordinates/areas are recomputed on
    host in float32 exactly as the reference does, so the selected
    outputs are bitwise-faithful to the reference semantics.

Layout per core: box b in [0, 42624) lives at partition p = b // 333,
free index f = b % 333, so both input rows (85 floats each) and output
rows are contiguous per partition for DMA.
"""

import numpy as np

import concourse.bass as bass
import concourse.bacc as bacc
import concourse.tile as tile
import concourse.mybir as mybir
from concourse.bass_utils import run_bass_kernel_spmd

F32 = mybir.dt.float32
I32 = mybir.dt.int32
ALU = mybir.AluOpType
AX = mybir.AxisListType

N_CORES = 8
P = 128

# problem constants (mirrors reference.py; kernel.py must be self-contained)
YOLO_INPUT_SIZE = np.float32(416.0)
SCORE_THRESHOLD = np.float32(0.3)
IOU_THRESHOLD = np.float32(0.45)
MAX_OUTPUT = 10

R416 = np.float32(np.float32(1.0) / np.float32(416.0))   # RN(1/416)
R416H = np.float32(R416 * np.float32(0.5))               # exact halving

NEG_BIG = np.float32(-131072.0)  # -2^17 rebias for argmax-index encoding


def _bcast_mid(ap, n):
    """[P, A] -> [P, n, A] with 0-stride middle dim."""
    return ap.unsqueeze(1).broadcast_to([ap.shape[0], n, ap.shape[1]])


def _bcast_last(ap, n):
    """[P, A] -> [P, A, n] with 0-stride last dim."""
    return ap.unsqueeze(2).broadcast_to([ap.shape[0], ap.shape[1], n])


_CUSTOM_OPS = {}


def _register_op(name, spec, subdim=False):
    """Register a runtime-defined custom DVE op (sha computed locally)."""
    import concourse.dve_ops as dve_ops_mod
    from concourse.dve_spec import lower as dve_lower, _has_src1
    from concourse.dve_uop import DveOpSpec

    if name in _CUSTOM_OPS:
        return _CUSTOM_OPS[name]
    shas = {}
    for ver in ("v3", "v4"):
        uops = dve_lower(spec, ver=ver)
        shas[ver] = DveOpSpec(name=name, opcode=None, uops=uops, rd1_en=True).sha(ver)
    op = dve_ops_mod.DveOp(name, spec, subdim=subdim, uops_sha=shas)
    if all(o.name != name for o in dve_ops_mod.OPS):
        dve_ops_mod.OPS.append(op)
        dve_ops_mod._SUB_OPCODE_FOR_NAME[name] = (
            dve_ops_mod._CUSTOM_DVE_ROW_BASE + len(dve_ops_mod.OPS) - 1
        )
    dve_ops_mod.CUSTOM_DVE_SPECS[name] = spec
    _CUSTOM_OPS[name] = op
    return op


def _register_decode_ops():
    """DECODE_LO: relu(in0*c0 - in1*c1); DECODE_HI: min(in0*c0 + in1*c1, 1).

    Each ALU stage rounds to f32, so these match the unfused op sequence
    bitwise."""
    from concourse.dve_spec import Spec, Src0, Src1, C0, C1, One, relu, minn

    r, rh = float(R416), float(R416H)

    def _lo_ref(in0, in1, c0, c1, c2):
        a = (np.asarray(in0, np.float32) * np.float32(c0)).astype(np.float32)
        b = (np.asarray(in1, np.float32) * np.float32(c1)).astype(np.float32)
        return np.maximum((a - b).astype(np.float32), np.float32(0.0))

    def _hi_ref(in0, in1, c0, c1, c2):
        a = (np.asarray(in0, np.float32) * np.float32(c0)).astype(np.float32)
        b = (np.asarray(in1, np.float32) * np.float32(c1)).astype(np.float32)
        return np.minimum((a + b).astype(np.float32), np.float32(1.0))

    lo = _register_op(
        "DECBOXLO_ANT",
        Spec(body=relu(Src0 * C0 - Src1 * C1), reference=_lo_ref),
    )
    hi = _register_op(
        "DECBOXHI_ANT",
        Spec(body=minn(Src0 * C0 + Src1 * C1, One), reference=_hi_ref),
    )
    return lo, hi


def _register_selgec():
    """Custom DVE op: out = (in0 >= in1) ? page-local index : s1.

    subdim op: in0/in1 are [P, S, N]; PageIdx(Zero, C0) holds 80*s (s0
    must be passed as float(N)), so Idx - pg is the within-page position.
    Output fits f16 exactly (values in [0, 79] or the fill 1024)."""
    from concourse.dve_spec import Spec, Src0, Src1, C0, C1, Zero, Idx, PageIdx, select

    def _ref(in0, in1, c0, c1, c2):
        in0 = np.asarray(in0, np.float32)
        assert in0.ndim == 3
        idx = np.arange(in0.shape[2], dtype=np.float32)[None, None, :]
        return np.where(
            in0 >= np.asarray(in1, np.float32), idx, np.float32(c1)
        ).astype(np.float32)

    spec = Spec(
        body=select(Src0 >= Src1, Idx - PageIdx(Zero, C0), C1), reference=_ref
    )
    return _register_op("SELGEC_ANT", spec, subdim=True)


def _register_selgecn():
    """Like SELGEC but negated: out = (in0 >= in1) ? -(page-local idx) : s1.

    max-pool over each 80-window then gives -(first argmax)."""
    from concourse.dve_spec import Spec, Src0, Src1, C0, C1, Zero, Idx, PageIdx, select

    def _ref(in0, in1, c0, c1, c2):
        in0 = np.asarray(in0, np.float32)
        assert in0.ndim == 3
        idx = -np.arange(in0.shape[2], dtype=np.float32)[None, None, :]
        return np.where(
            in0 >= np.asarray(in1, np.float32), idx, np.float32(c1)
        ).astype(np.float32)

    spec = Spec(
        body=select(Src0 >= Src1, PageIdx(Zero, C0) - Idx, C1), reference=_ref
    )
    return _register_op("SELGECN_ANT", spec, subdim=True)


def _register_selgeidx():
    """Custom DVE op: out = (in0 >= in1) ? stream_index : s1.

    Fuses the mask + masked-index passes of the per-box argmax into one
    VectorE instruction.  s1 (C1) is a compile-time float (required when
    in1 has two free dims).
    """
    from concourse.dve_spec import Spec, Src0, Src1, C1, Idx, select

    def _ref(in0, in1, c0, c1, c2):
        in0 = np.asarray(in0, np.float32)
        fshape = in0.shape[1:]
        idx = np.arange(int(np.prod(fshape)), dtype=np.float32).reshape(fshape)
        return np.where(
            in0 >= np.asarray(in1, np.float32), idx[None], np.float32(c1)
        ).astype(np.float32)

    return _register_op(
        "SELGEIDX_ANT", Spec(body=select(Src0 >= Src1, Idx, C1), reference=_ref)
    )


def build_program(
    fpad=333, fg=37, n_cores=N_CORES, use_custom=True, bufs=(3, 2, 3),
    argmax_mode="selgec",
):
    """Build + compile the SPMD Bass program (one NeuronCore's view).

    fpad:     boxes per partition (per core total = 128*fpad)
    fg:       boxes per partition processed per tile (fpad % fg == 0)
    use_custom: fuse passes into custom DVE ops
    argmax_mode: "selgec" (page-local f16 + tree-min) or "selgeidx"
    """
    assert fpad % fg == 0
    ntiles = fpad // fg
    bpad = P * fpad
    F16 = mybir.dt.float16
    if not use_custom:
        argmax_mode = "stock"
    sel_op = _register_selgeidx() if use_custom else None
    selc_op = _register_selgec() if use_custom else None
    selcn_op = _register_selgecn() if use_custom else None
    lo_op, hi_op = _register_decode_ops() if use_custom else (None, None)

    nc = bacc.Bacc(
        "TRN2",
        target_bir_lowering=False,
        debug=False,
        num_devices=n_cores,
    )

    yp = nc.dram_tensor("yp", [bpad, 85], F32, kind="ExternalInput")
    boxes_o = nc.dram_tensor("boxes", [bpad, 4], F32, kind="ExternalOutput")
    scores_o = nc.dram_tensor("scores", [bpad], F32, kind="ExternalOutput")
    classes_o = nc.dram_tensor("classes", [bpad], I32, kind="ExternalOutput")

    ypr = yp.ap().rearrange("(p f) c -> p f c", p=P)
    boxr = boxes_o.ap().rearrange("(p f) c -> p f c", p=P)
    scorer = scores_o.ap().rearrange("(p f) -> p f", p=P)
    classr = classes_o.ap().rearrange("(p f) -> p f", p=P)

    from contextlib import ExitStack

    with tile.TileContext(nc) as tc, ExitStack() as ctx:
        persist = ctx.enter_context(tc.tile_pool(name="persist", bufs=1))
        inpool = ctx.enter_context(tc.tile_pool(name="inp", bufs=bufs[0]))
        maskpool = ctx.enter_context(tc.tile_pool(name="mask", bufs=bufs[1]))
        small = ctx.enter_context(tc.tile_pool(name="small", bufs=bufs[2]))
        dec = ctx.enter_context(tc.tile_pool(name="dec", bufs=3))

        # persistent result tiles (DMA'd out once at the end)
        boxes_sb = persist.tile([P, fpad, 4], F32)
        scores_sb = persist.tile([P, fpad], F32)
        classes_sb = persist.tile([P, fpad], I32)
        negc_sb = None
        if argmax_mode == "selgecpool":
            negc_sb = persist.tile([P, fpad], F16, name="negc_sb")
        cmin_sb = None
        if argmax_mode == "selgec":
            cmin_sb = persist.tile([P, fpad], F16, name="cmin_sb")

        # constants (f32 iota is exact for |v| < 2^24)
        # bigiota[c] = c - 2^17: mask*bigiota -> min = argmax - 2^17
        bigiota = persist.tile([P, 80], F32)
        nc.gpsimd.iota(
            bigiota[:], pattern=[[1, 80]], base=-131072, channel_multiplier=0,
            allow_small_or_imprecise_dtypes=True,
        )
        # col80[j] = 80*j for j in [0, fg): recovers the class from the
        # custom op's per-instruction stream index (local to each tile)
        col80 = persist.tile([P, fg], F32)
        nc.gpsimd.iota(
            col80[:], pattern=[[80, fg]], base=0, channel_multiplier=0,
            allow_small_or_imprecise_dtypes=True,
        )

        for t in range(ntiles):
            sl = slice(t * fg, (t + 1) * fg)

            bt = inpool.tile([P, fg, 85], F32)
            nc.sync.dma_start(out=bt[:], in_=ypr[:, sl, :])

            probs = bt[:, :, 5:85]
            conf = bt[:, :, 4]

            # ---- class max + scores ----
            m = small.tile([P, fg], F32, tag="m")
            nc.vector.reduce_max(m[:], probs, axis=AX.X)
            nc.vector.tensor_tensor(
                out=scores_sb[:, sl], in0=conf, in1=m[:], op=ALU.mult
            )

            # ---- first-occurrence argmax over 80 classes ----
            if argmax_mode == "selgecpool":
                # fused pass -> negated page-local index in f16, then ONE
                # max-pool (window 80) per tile; classes converted in one
                # batched op after the loop
                # inner stride 81 keeps [fg, 80] from coalescing so pool
                # sees the 80-wide window as its innermost dim
                mk16 = maskpool.tile([P, fg, 81], F16, tag="mk16")
                nc.vector._custom_dve(
                    selcn_op, out=mk16[:, :, 0:80], in0=probs,
                    in1=_bcast_last(m[:], 80), s0=80.0, s1=-1024.0,
                )
                nc.vector.pool_max(negc_sb[:, sl], mk16[:, :, 0:80])
            elif argmax_mode == "selgec":
                # fused pass -> page-local index in f16, then a 2x-mode
                # f16 min-tree (cheaper than the 1x tensor_reduce)
                mk16 = maskpool.tile([P, fg, 80], F16, tag="mk16")
                nc.vector._custom_dve(
                    selc_op, out=mk16[:], in0=probs,
                    in1=_bcast_last(m[:], 80), s0=80.0, s1=1024.0,
                )
                cur = mk16[:]
                for w in (40, 20, 10):
                    dst = dec.tile([P, fg, w], F16, tag=f"tm{w}")
                    nc.vector.tensor_tensor(
                        out=dst[:], in0=cur[:, :, 0:w], in1=cur[:, :, w:2 * w],
                        op=ALU.min,
                    )
                    cur = dst[:]
                # per-tile result goes to a persistent f16 row; ONE batched
                # f16->i32 conversion happens after the loop
                nc.vector.tensor_reduce(
                    cmin_sb[:, sl], cur, axis=AX.X, op=ALU.min
                )
                mk = None
                tmin = None
            else:
                mk = maskpool.tile([P, fg, 80], F32, tag="mk")
                tmin = small.tile([P, fg], F32, tag="tmin")
            if argmax_mode in ("selgec", "selgecpool"):
                pass
            elif use_custom:
                # one fused pass: idx where p >= m else BIG
                nc.vector._custom_dve(
                    sel_op, out=mk[:], in0=probs,
                    in1=_bcast_last(m[:], 80), s1=1.0e9,
                )
                nc.vector.tensor_reduce(tmin[:], mk[:], axis=AX.X, op=ALU.min)
                # class = gmin - 80*f  (exact ints in f32; i32 on write)
                nc.vector.tensor_tensor(
                    out=classes_sb[:, sl], in0=tmin[:], in1=col80[:],
                    op=ALU.subtract,
                )
            else:
                nc.vector.tensor_tensor(
                    out=mk[:], in0=probs, in1=_bcast_last(m[:], 80),
                    op=ALU.is_ge,
                )
                nc.vector.tensor_tensor(
                    out=mk[:], in0=mk[:], in1=_bcast_mid(bigiota[:], fg),
                    op=ALU.mult,
                )
                nc.vector.tensor_reduce(tmin[:], mk[:], axis=AX.X, op=ALU.min)
                nc.vector.tensor_scalar_add(
                    out=classes_sb[:, sl], in0=tmin[:], scalar1=float(-NEG_BIG)
                )

            # ---- box decode ----
            if use_custom:
                nc.vector._custom_dve(
                    lo_op, out=boxes_sb[:, sl, 0:2], in0=bt[:, :, 0:2],
                    in1=bt[:, :, 2:4], s0=float(R416), s1=float(R416H),
                )
                nc.vector._custom_dve(
                    hi_op, out=boxes_sb[:, sl, 2:4], in0=bt[:, :, 0:2],
                    in1=bt[:, :, 2:4], s0=float(R416), s1=float(R416H),
                )
            else:
                xy = dec.tile([P, fg, 2], F32, tag="xy")
                wh = dec.tile([P, fg, 2], F32, tag="wh")
                nc.scalar.mul(xy[:], bt[:, :, 0:2], float(R416))
                nc.scalar.mul(wh[:], bt[:, :, 2:4], float(R416H))
                t1 = dec.tile([P, fg, 2], F32, tag="t1")
                t2 = dec.tile([P, fg, 2], F32, tag="t2")
                nc.vector.tensor_sub(t1[:], xy[:], wh[:])
                nc.vector.tensor_scalar_max(
                    out=boxes_sb[:, sl, 0:2], in0=t1[:], scalar1=0.0
                )
                nc.vector.tensor_add(t2[:], xy[:], wh[:])
                nc.vector.tensor_scalar_min(
                    out=boxes_sb[:, sl, 2:4], in0=t2[:], scalar1=1.0
                )

            # stream this tile's boxes/scores out while later tiles compute
            nc.sync.dma_start(out=boxr[:, sl, :], in_=boxes_sb[:, sl, :])
            nc.sync.dma_start(out=scorer[:, sl], in_=scores_sb[:, sl])

        if argmax_mode == "selgecpool":
            # classes = -negc (batched; exact ints in f16 -> i32 on write)
            nc.vector.tensor_scalar_mul(
                out=classes_sb[:], in0=negc_sb[:], scalar1=-1.0
            )
        elif argmax_mode == "selgec":
            # batched f16 -> i32 conversion (exact integers)
            nc.vector.tensor_scalar_add(
                out=classes_sb[:], in0=cmin_sb[:], scalar1=0.0
            )
        nc.sync.dma_start(out=classr[:], in_=classes_sb[:])

    nc.compile()
    return nc


# ---------------------------------------------------------------------------
# host-side NMS tail (bitwise-faithful to reference semantics in float32)
# ---------------------------------------------------------------------------

def _decode_boxes_np(raw):
    b = np.clip(raw / YOLO_INPUT_SIZE, np.float32(0.0), np.float32(1.0)).astype(
        np.float32
    )
    x, y, w, h = b[:, 0], b[:, 1], b[:, 2], b[:, 3]
    half = np.float32(0.5)
    one = np.float32(1.0)
    zero = np.float32(0.0)
    x1 = np.clip(x - half * w, zero, one)
    y1 = np.clip(y - half * h, zero, one)
    x2 = np.clip(x + half * w, zero, one)
    y2 = np.clip(y + half * h, zero, one)
    return np.stack([x1, y1, x2, y2], axis=-1).astype(np.float32)


def _nms_on_subset(cand_idx, boxes_c, scores_c, cutoff, n_total):
    """Run the reference NMS restricted to candidate boxes.

    cand_idx: global indices (int64) of candidates, boxes_c [K,4] f32,
    scores_c [K] f32 (exact reference-path values), cutoff: min original
    score over candidates (f32).  Returns (sel_idx[10] i32,
    sel_scores[10] f32, ok flag).  ok=False => guard failed, caller must
    fall back to the full computation.
    """
    # order candidates by global index so np.argmax tie-breaks identically
    order = np.argsort(cand_idx, kind="stable")
    cand_idx = cand_idx[order]
    boxes_c = boxes_c[order]
    scores_c = scores_c[order]

    x1, y1, x2, y2 = boxes_c[:, 0], boxes_c[:, 1], boxes_c[:, 2], boxes_c[:, 3]
    areas = ((x2 - x1) * (y2 - y1)).astype(np.float32)
    neg_inf = np.float32(-np.inf)
    sw = np.where(scores_c >= SCORE_THRESHOLD, scores_c, neg_inf).astype(np.float32)

    sel_idx = np.full(MAX_OUTPUT, -1, np.int32)
    sel_sc = np.zeros(MAX_OUTPUT, np.float32)
    for i in range(MAX_OUTPUT):
        j = int(np.argmax(sw))
        valid = np.isfinite(sw[j])
        if not valid:
            # all remaining -inf; matches reference (idx -1, score 0).
            # Guard: the true NMS could still have valid boxes outside the
            # candidate set only if cutoff >= threshold.
            if cutoff >= SCORE_THRESHOLD:
                return sel_idx, sel_sc, False
            continue
        # guard: winner must be strictly above every non-candidate score
        if not (sw[j] > cutoff):
            return sel_idx, sel_sc, False
        sel_idx[i] = np.int32(cand_idx[j])
        sel_sc[i] = scores_c[j]
        iw = np.maximum(
            np.minimum(x2, x2[j]) - np.maximum(x1, x1[j]), np.float32(0.0)
        ).astype(np.float32)
        ih = np.maximum(
            np.minimum(y2, y2[j]) - np.maximum(y1, y1[j]), np.float32(0.0)
        ).astype(np.float32)
        inter = (iw * ih).astype(np.float32)
        union = np.maximum(
            (areas + areas[j]).astype(np.float32) - inter, np.float32(1e-9)
        ).astype(np.float32)
        iou = (inter / union).astype(np.float32)
        sw = np.where(np.isfinite(sw[j]) & (iou > IOU_THRESHOLD), neg_inf, sw)
        sw[j] = neg_inf
    return sel_idx, sel_sc, True


def _host_nms_tail(flat, scores_full, topk=4096):
    """flat: y_pred reshaped [N, 85] f32; scores_full [N] f32 (device).

    Recomputes candidate boxes/scores exactly as the reference does
    (true f32 division, clips) so selected outputs match bitwise.
    """
    n = flat.shape[0]
    k = min(topk, n)
    cand = np.argpartition(scores_full, n - k)[n - k:]
    cutoff = np.float32(scores_full[cand].min())

    rows = flat[cand]
    boxes_c = _decode_boxes_np(rows[:, :4])
    scores_c = (
        rows[:, 4].astype(np.float32)
        * np.max(rows[:, 5:], axis=-1).astype(np.float32)
    ).astype(np.float32)

    global LAST_NMS_FALLBACK
    sel_idx, sel_sc, ok = _nms_on_subset(
        cand.astype(np.int64), boxes_c, scores_c, cutoff, n
    )
    LAST_NMS_FALLBACK = not ok
    if ok:
        return sel_idx, sel_sc

    # fallback: exact full-N NMS on host (never expected to trigger)
    boxes_f = _decode_boxes_np(flat[:, :4])
    scores_f = (
        flat[:, 4].astype(np.float32)
        * np.max(flat[:, 5:], axis=-1).astype(np.float32)
    ).astype(np.float32)
    sel_idx, sel_sc, ok = _nms_on_subset(
        np.arange(n, dtype=np.int64), boxes_f, scores_f, np.float32(-np.inf), n
    )
    assert ok
    return sel_idx, sel_sc


# ---------------------------------------------------------------------------
# public entry point
# ---------------------------------------------------------------------------

_NC_CACHE = {}
LAST_NMS_FALLBACK = False


def _get_program():
    key = "main"
    if key not in _NC_CACHE:
        _NC_CACHE[key] = build_program()
    return _NC_CACHE[key]


def run_device(flat: np.ndarray, trace: bool = False, **kwargs):
    """Run the SPMD device program on the flattened [N, 85] input.

    Returns (boxes [N,4], scores [N], classes [N], BassKernelResults).
    """
    n = flat.shape[0]
    per_core = n // N_CORES                         # 42588
    fpad = 333
    bpad = P * fpad                                 # 42624

    nc = _get_program()

    in_maps = []
    for c in range(N_CORES):
        shard = flat[c * per_core:(c + 1) * per_core]
        if bpad != per_core:
            pad = np.zeros((bpad - per_core, 85), np.float32)
            shard = np.concatenate([shard, pad], axis=0)
        in_maps.append({"yp": np.ascontiguousarray(shard)})

    res = run_bass_kernel_spmd(
        nc, in_maps, core_ids=list(range(N_CORES)), trace=trace, **kwargs
    )
    results = res.results

    boxes = np.concatenate(
        [results[c]["boxes"][:per_core] for c in range(N_CORES)], axis=0
    ).astype(np.float32)
    scores = np.concatenate(
        [results[c]["scores"][:per_core] for c in range(N_CORES)], axis=0
    ).astype(np.float32)
    classes = np.concatenate(
        [results[c]["classes"][:per_core] for c in range(N_CORES)], axis=0
    ).astype(np.int32)
    return boxes, scores, classes, res


def kernel(y_pred: np.ndarray):
    y_pred = np.asarray(y_pred, dtype=np.float32)
    n = y_pred.shape[0] * y_pred.shape[1]          # 340704
    flat = np.ascontiguousarray(y_pred.reshape(n, y_pred.shape[-1]))

    boxes, scores, classes, _ = run_device(flat)

    sel_idx, sel_sc = _host_nms_tail(flat, scores)

    return (
        boxes,
        scores[:, None],
        classes[:, None],
        sel_idx,
        sel_sc,
    )


if __name__ == "__main__":
    rng = np.random.default_rng(0)
    y = rng.random((32, 10647, 85), dtype=np.float32) * np.array(
        [416.0] * 4 + [1.0] * 81, np.float32
    )
    out = kernel(y_pred=y)
    for o in out:
        print(o.shape, o.dtype)
